# revision 10
# baseline (speedup 1.0000x reference)
"""Trainium2 Bass kernel for a dense transformer block (LN -> QKV -> attention ->
out-proj -> LN -> FFN with exact GELU, no residuals).

Sharding: pure data parallelism — batch 8 across 8 NeuronCores, one batch element
per core. Each core runs the full block on its [1024, 1024] token slab.

On-chip dataflow (per core):
  - LN1 in token-major fp32 (bn_stats); normalized output cast to bf16 and
    transposed to feature-major xnT [D, tokens] on the PE (bf16 transpose-mode).
  - All GEMMs in bf16 with fp32 PSUM accumulation. LayerNorm gains fold into the
    weights on the host; LN biases fold into per-feature GEMM biases. The
    1/sqrt(dh) attention scale folds into the Q projection.
  - Q/K produced feature-major per head pair (streamed), V token-major with a
    ones-column per head so attention@V also yields the softmax denominator.
  - Scores are computed k-major (scoresT) so the exp output feeds attention@V
    directly with no transpose; softmax skips max-subtraction (|scores| < ~3).

Scheduling (the perf-critical part — the PE must never idle >3.4us or the HAM
clock gate re-throttles it to 1.2 GHz):
  - DMA priority order at start: x (8 per-tile chunks, so LN1 starts ~1us in and
    the PE transposes keep HAM warm from the start) -> wv -> wq -> wk -> wo.
    All sync-queue DMAs share one HW queue in issue order, ~280 GB/s aggregate.
  - Attention is a single software-pipelined stream per head-pair hp of 32
    "slots" (one score MM each). Interleaved into the slots: the av MMs (lag 4
    behind their exp), the QK chain MMs for hp+1, the denominator finalize for
    the chains that completed (copy den row -> bf16 ones-matmul partition
    broadcast -> DVE reciprocal -> aoT multiply). The PE queue never waits on
    the DVE chain because every consumer sits >=2 slots after its producer.
  - PSUM tags: sc 2x[128,512] (scores/transposes/psb ring), qc 1x[128,1024]
    (QK chains, WAW-rotated), us 4x[128,512] (av accumulators, FFN2
    accumulators) = exactly 8 banks.
  - FFN2 runs in 4 groups of 2 token tiles so output stores overlap compute and
    only ~2us of final DMA is exposed.
"""

import numpy as np
import ml_dtypes
from collections import deque

B, N, D = 8, 1024, 1024
H, DH = 16, 64
MLP = 4096
EPS = 1e-5
P = 128
NCORES = 8
TT = N // P    # 8 token tiles
DC = D // P    # 8 d-chunks
MT = MLP // P  # 32 mlp tiles


def build_bass(gelu_mode="gelu"):
    import concourse.bass as bass
    import concourse.mybir as mybir
    import concourse.tile as tile
    from concourse import bacc
    from concourse.masks import make_identity

    f32 = mybir.dt.float32
    bf16 = mybir.dt.bfloat16
    AF = mybir.ActivationFunctionType
    OP = mybir.AluOpType

    nc = bacc.Bacc()

    x_d = nc.declare_dram_parameter("x", [N, D], bf16, isOutput=False)
    wq_d = nc.declare_dram_parameter("wq", [D, D], bf16, isOutput=False)
    wk_d = nc.declare_dram_parameter("wk", [D, D], bf16, isOutput=False)
    wv_d = nc.declare_dram_parameter("wv", [D, D], bf16, isOutput=False)
    wo_d = nc.declare_dram_parameter("wo", [D, D], bf16, isOutput=False)
    w1_d = nc.declare_dram_parameter("w1", [D, MLP], bf16, isOutput=False)
    w2_d = nc.declare_dram_parameter("w2", [MLP, D], bf16, isOutput=False)
    bq_d = nc.declare_dram_parameter("bq", [D], f32, isOutput=False)
    bk_d = nc.declare_dram_parameter("bk", [D], f32, isOutput=False)
    bv_d = nc.declare_dram_parameter("bv", [D], f32, isOutput=False)
    bo_d = nc.declare_dram_parameter("bo", [D], f32, isOutput=False)
    bh_d = nc.declare_dram_parameter("bh", [MLP], f32, isOutput=False)
    b2_d = nc.declare_dram_parameter("b2", [D], f32, isOutput=False)
    out_d = nc.declare_dram_parameter("out", [N, D], f32, isOutput=True)

    gelu_func = AF.Gelu if gelu_mode == "gelu" else AF.Identity

    with tile.TileContext(nc) as tc:
        # ---- permanent pools (left stack bottom) ----
        const = tc.alloc_tile_pool(name="const", bufs=1)
        stats = tc.alloc_tile_pool(name="stats", bufs=4)
        psum = tc.alloc_tile_pool(name="psum", bufs=2, space="PSUM")
        wslot = tc.alloc_tile_pool(name="wslot", bufs=4)   # 4 x 16KB weight slots
        outp = tc.alloc_tile_pool(name="outp", bufs=2)

        counter = [0]

        def uniq(prefix):
            counter[0] += 1
            return f"{prefix}{counter[0]}"

        def sc_tile(shape=None, dtype=None):
            return psum.tile(shape or [P, 512], dtype or f32, tag="sc", bufs=4,
                             name=uniq("sc"))

        def us_tile():
            return psum.tile([P, 512], f32, tag="us", bufs=2, name=uniq("us"))

        def qc_tile():
            return psum.tile([P, 1024], f32, tag="qc", bufs=1, name=uniq("qc"))

        def wtile(shape):
            return wslot.tile(shape, bf16, tag="w", name=uniq("w"))

        eps_t = const.tile([P, 1], f32, tag="eps")
        nc.vector.memset(eps_t, EPS)
        bq_sb = const.tile([P, DC], f32, tag="bq")
        nc.sync.dma_start(bq_sb, bq_d[:].rearrange("(o p) -> p o", p=P))
        bk_sb = const.tile([P, DC], f32, tag="bk")
        nc.sync.dma_start(bk_sb, bk_d[:].rearrange("(o p) -> p o", p=P))
        bh_sb = const.tile([P, MT], f32, tag="bh")
        nc.sync.dma_start(bh_sb, bh_d[:].rearrange("(o p) -> p o", p=P))
        ident = const.tile([P, P], bf16, tag="ident")
        make_identity(nc, ident)
        ones64b = const.tile([1, 64], bf16, tag="ones64b")
        nc.vector.memset(ones64b, 1.0)

        # ---- phase 1 pools ----
        lnxn = tc.alloc_tile_pool(name="lnxn", bufs=2)     # dies after LN2
        aop = tc.alloc_tile_pool(name="ao", bufs=1)
        uscp = tc.alloc_tile_pool(name="usc", bufs=4)
        denbp = tc.alloc_tile_pool(name="denb", bufs=2)
        x2p = tc.alloc_tile_pool(name="x2", bufs=2)
        xnTp = tc.alloc_tile_pool(name="xnT", bufs=1)
        vap = tc.alloc_tile_pool(name="vaug", bufs=1)
        qkp = tc.alloc_tile_pool(name="qk", bufs=4)
        expp = tc.alloc_tile_pool(name="expp", bufs=8)
        lnx = tc.alloc_tile_pool(name="lnx", bufs=1, side="right")

        # x first among the big DMAs (one chunk per token tile so LN1 can
        # start ~1us in), then weights in order of first use.
        xfull = lnx.tile([P, TT, D], bf16, tag="x")
        for ti in range(TT):
            nc.sync.dma_start(xfull[:, ti, :], x_d[ti * P:(ti + 1) * P, :])
        wv_sb = wtile([P, DC, D])
        nc.sync.dma_start(wv_sb, wv_d[:, :].rearrange("(o p) f -> p o f", p=P))
        wq_sb = wtile([P, DC, D])
        nc.sync.dma_start(wq_sb, wq_d[:, :].rearrange("(o p) f -> p o f", p=P))
        wk_sb = wtile([P, DC, D])
        nc.sync.dma_start(wk_sb, wk_d[:, :].rearrange("(o p) f -> p o f", p=P))
        wo_sb = wtile([P, DC, D])
        nc.sync.dma_start(wo_sb, wo_d[:, :].rearrange("(o p) f -> p o f", p=P))

        # broadcast bias tiles on the gpsimd software queue (parallel path)
        bv_b = const.tile([P, D], bf16, tag="bvb")
        nc.gpsimd.dma_start(bv_b, bv_d[None, :].to_broadcast([P, D]))
        bo_b = const.tile([P, D], bf16, tag="bob")
        nc.gpsimd.dma_start(bo_b, bo_d[None, :].to_broadcast([P, D]))
        b2_b = const.tile([P, D], bf16, tag="b2b")
        nc.gpsimd.dma_start(b2_b, b2_d[None, :].to_broadcast([P, D]))

        def layer_norm_tile(x_t, xn_t):
            """token-major [128, D] -> normalized bf16 (no gain/bias)."""
            nc.vector.memset(xn_t[0:1, 0:4], 0.0)   # claim slot: absorb WAR deps
            st = stats.tile([P, 2, 6], f32, tag="st", name=uniq("st"))
            xr = x_t.rearrange("p (s d) -> p s d", s=2)
            nc.vector.bn_stats(st[:, 0], xr[:, 0])
            nc.vector.bn_stats(st[:, 1], xr[:, 1])
            mv = stats.tile([P, 2], f32, tag="mv", name=uniq("mv"))
            nc.vector.bn_aggr(mv, st)
            rstd = stats.tile([P, 1], f32, tag="rstd", name=uniq("rstd"))
            nc.scalar.activation(rstd, mv[:, 1:2], func=AF.Sqrt, bias=eps_t,
                                 scale=1.0)
            nc.vector.reciprocal(rstd, rstd)
            nc.vector.tensor_scalar(xn_t, x_t, scalar1=mv[:, 0:1], scalar2=rstd,
                                    op0=OP.subtract, op1=OP.mult)

        def pe_transpose_to(dst_of_dj, src_t):
            """[128 tok, D] bf16 -> feature-major dst[:, dj, tok-slice]."""
            for dj in range(DC):
                pst = sc_tile([P, P], bf16)
                nc.tensor.transpose(pst, src_t[:, dj * P:(dj + 1) * P], ident)
                nc.scalar.activation(dst_of_dj(dj), pst, func=AF.Copy)

        # ---- phase 1: LN1 + transpose to feature-major; V fused one behind ----
        xnT = xnTp.tile([P, DC, N], bf16, tag="xnT")
        v_aug = vap.tile([P, TT, H, DH + 1], bf16, tag="vaug")
        nc.vector.memset(v_aug[:, :, :, DH:DH + 1], 1.0)

        def emit_v(ti):
            for fh in range(2):
                psv = us_tile()
                for dc in range(DC):
                    nc.tensor.matmul(psv, lhsT=xnT[:, dc, ti * P:(ti + 1) * P],
                                     rhs=wv_sb[:, dc, fh * 512:(fh + 1) * 512],
                                     start=(dc == 0), stop=(dc == DC - 1))
                nc.vector.tensor_tensor(
                    v_aug[:, ti, fh * 8:(fh + 1) * 8, 0:DH],
                    psv[:].rearrange("p (h d) -> p h d", d=DH),
                    bv_b[:, fh * 512:(fh + 1) * 512].rearrange(
                        "p (h d) -> p h d", d=DH),
                    OP.add)

        for ti in range(TT):
            xn_t = lnxn.tile([P, D], bf16, tag="xn", name=uniq("xn"))
            layer_norm_tile(xfull[:, ti, :], xn_t)
            pe_transpose_to(
                lambda dj, ti=ti: xnT[:, dj, ti * P:(ti + 1) * P], xn_t)
            if ti > 0:
                emit_v(ti - 1)
        emit_v(TT - 1)
        lnx.release()

        # ---- phase 2: attention, software-pipelined slot stream ----
        # Per head-pair hp: 32 score slots (one [128,512] score MM + exp each).
        # The av matmuls trail their exp by LAG slots; the qh0 and qh1 av
        # chains time-share 2 PSUM accumulator banks (qh0 avs end at slot 21,
        # qh1 avs start at slot 22) with a DVE drain to SBUF between tenants.
        # The denominator finalize (bf16 ones-matmul partition broadcast ->
        # reciprocal -> aoT multiply) runs off the SBUF copies, so the PE
        # never waits on it. QK chains for hp+1 are emitted as one dense
        # 6.8us burst at slot 16 — that burst also re-warms the HAM clock
        # gate every iteration if anything throttled the PE.
        aoT = aop.tile([P, DC, N], bf16, tag="aoT")

        qt_of = {}   # hp -> (qt, kt) SBUF tiles
        us_of = {}   # (hp, hh) -> psum accumulator (qh0 then qh1)
        usc_of = {}  # (hp, hh, qh) -> SBUF bf16 copy [DH+1, 512]
        qc_of = {}   # hp -> qc psum tile for the QK chains

        def alloc_qk(hp):
            qt = qkp.tile([P, N], bf16, tag="qT", name=uniq("qT"))
            kt = qkp.tile([P, N], bf16, tag="kT", name=uniq("kT"))
            nc.vector.memset(qt[0:1, 0:4], 0.0)
            nc.vector.memset(kt[0:1, 0:4], 0.0)
            qt_of[hp] = (qt, kt)
            qc_of[hp] = qc_tile()

        # chain c: 0=q/qh0, 1=k/qh0, 2=q/qh1, 3=k/qh1. qc[:, 0:512] holds q
        # chains, qc[:, 512:1024] k chains (WAW rotated after the DVE add).
        def emit_qk_block(hp):
            qt, kt = qt_of[hp]
            qc = qc_of[hp]
            for c in range(4):
                w_sb = wq_sb if c % 2 == 0 else wk_sb
                qh = c // 2
                half = qc[:, (c % 2) * 512:(c % 2) * 512 + 512]
                for j in range(DC):
                    nc.tensor.matmul(half,
                                     lhsT=w_sb[:, j, hp * P:(hp + 1) * P],
                                     rhs=xnT[:, j, qh * 512:(qh + 1) * 512],
                                     start=(j == 0), stop=(j == DC - 1))
                dst = qt if c % 2 == 0 else kt
                bias = bq_sb if c % 2 == 0 else bk_sb
                nc.vector.tensor_scalar_add(dst[:, qh * 512:(qh + 1) * 512],
                                            half, bias[:, hp:hp + 1])

        pend = deque()   # (hp, hh, qh, mc, ext)
        LAG = 6

        def flush_av():
            hp_, hh, qh, mc, ext = pend.popleft()
            nc.tensor.matmul(us_of[(hp_, hh)][0:DH + 1, :],
                             lhsT=v_aug[:, mc, 2 * hp_ + hh, :],
                             rhs=ext,
                             start=(mc == 0), stop=(mc == TT - 1))

        def drain_us(hp, qh):
            for hh in range(2):
                u = uscp.tile([DH, 512], bf16, tag="usc", name=uniq("usc"))
                nc.vector.tensor_copy(u, us_of[(hp, hh)][0:DH, :])
                dn = uscp.tile([1, 512], bf16, tag="den", name=uniq("den"))
                nc.vector.tensor_copy(dn, us_of[(hp, hh)][DH:DH + 1, :])
                usc_of[(hp, hh, qh)] = (u, dn)

        def fin(hp, qh):
            for hh in range(2):
                u, dn = usc_of.pop((hp, hh, qh))
                psb = sc_tile()
                nc.tensor.matmul(psb[0:64, :], lhsT=ones64b[:],
                                 rhs=dn, start=True, stop=True)
                dB = denbp.tile([64, 512], f32, tag="dB", name=uniq("dB"))
                nc.vector.reciprocal(dB, psb[0:64, :])
                nc.vector.tensor_mul(
                    aoT[64 * hh:64 * hh + 64, hp, qh * 512:(qh + 1) * 512],
                    u, dB)

        # pre-loop: QK chains for hp=0 run right after the V matmuls
        alloc_qk(0)
        emit_qk_block(0)

        score_order = [(qh, mc, hh) for qh in range(2) for mc in range(TT)
                       for hh in range(2)]

        for hp in range(DC):
            us_of[(hp, 0)] = us_tile()
            us_of[(hp, 1)] = us_tile()
            if hp < DC - 1:
                alloc_qk(hp + 1)
            qt, kt = qt_of[hp]
            for s, (qh, mc, hh) in enumerate(score_order):
                r0 = 64 * hh
                sp = sc_tile()
                nc.tensor.matmul(sp,
                                 lhsT=kt[r0:r0 + 64, mc * P:(mc + 1) * P],
                                 rhs=qt[r0:r0 + 64, qh * 512:(qh + 1) * 512],
                                 start=True, stop=True)
                ext = expp.tile([P, 512], bf16, tag="expT", name=uniq("expT"))
                nc.scalar.activation(ext, sp, func=AF.Exp)
                pend.append((hp, hh, qh, mc, ext))
                if s == 6 and hp > 0:
                    drain_us(hp - 1, 1)
                if s == 8 and hp > 0:
                    fin(hp - 1, 1)
                if s == 16 and hp < DC - 1:
                    emit_qk_block(hp + 1)
                if s == 22:
                    drain_us(hp, 0)
                if s == 24:
                    fin(hp, 0)
                while len(pend) > LAG:
                    flush_av()
        while pend:
            flush_av()
        drain_us(DC - 1, 1)
        fin(DC - 1, 1)

        expp.release()
        qkp.release()
        vap.release()
        xnTp.release()

        # q/k/v slots free -> start w1 loads
        w1_t = []
        for g in range(3):   # quarters 0-2 reuse the q/k/v slots right away
            t = wtile([P, 2, MLP])
            nc.sync.dma_start(
                t, w1_d[g * 256:(g + 1) * 256, :].rearrange(
                    "(o p) f -> p o f", p=P))
            w1_t.append(t)

        # ---- phase 4: out-projection + LN2 + transpose ----
        hTp = tc.alloc_tile_pool(name="hT", bufs=1, side="right")
        xn2Tp = tc.alloc_tile_pool(name="xn2T", bufs=1, side="right")
        hT = hTp.tile([P, MT, N], bf16, tag="hT")
        xn2T = xn2Tp.tile([P, DC, N], bf16, tag="xn2T")

        pend_tr = [None]
        for ti in range(TT):
            x2_t = x2p.tile([P, D], bf16, tag="x2", name=uniq("x2"))
            nc.vector.memset(x2_t[0:1, 0:4], 0.0)
            for fh in range(2):
                pso = sc_tile()
                for dc in range(DC):
                    nc.tensor.matmul(pso, lhsT=aoT[:, dc, ti * P:(ti + 1) * P],
                                     rhs=wo_sb[:, dc, fh * 512:(fh + 1) * 512],
                                     start=(dc == 0), stop=(dc == DC - 1))
                nc.vector.tensor_add(x2_t[:, fh * 512:(fh + 1) * 512], pso,
                                     bo_b[:, fh * 512:(fh + 1) * 512])
            # defer this tile's transposes behind the next tile's out-proj so
            # the in-order PE queue never waits on the DVE LayerNorm chain
            if pend_tr[0] is not None:
                pend_tr[0]()
            xn2_t = lnxn.tile([P, D], bf16, tag="xn", name=uniq("xn"))
            layer_norm_tile(x2_t, xn2_t)
            pend_tr[0] = (lambda ti=ti, xt=xn2_t: pe_transpose_to(
                lambda dj: xn2T[:, dj, ti * P:(ti + 1) * P], xt))
        pend_tr[0]()

        # wout done -> load the last w1 quarter into its slot
        w1d = wtile([P, 2, MLP])
        nc.sync.dma_start(
            w1d, w1_d[768:1024, :].rearrange("(o p) f -> p o f", p=P))
        w1_t.append(w1d)

        x2p.release()
        denbp.release()
        uscp.release()
        aop.release()
        lnxn.release()
        w2ep = tc.alloc_tile_pool(name="w2e", bufs=2)
        w2_t = []
        for g in range(2):
            t = w2ep.tile([P, TT, D], bf16, tag="w2e", name=uniq("w2e"))
            nc.sync.dma_start(
                t, w2_d[g * 1024:(g + 1) * 1024, :].rearrange(
                    "(o p) f -> p o f", p=P))
            w2_t.append(t)

        # ---- phase 5: FFN1 (feature-major h, fused bias+gelu) ----
        for m in range(MT):
            for qh in range(2):
                psh = sc_tile()
                for dc in range(DC):
                    nc.tensor.matmul(
                        psh,
                        lhsT=w1_t[dc // 2][:, dc % 2, m * P:(m + 1) * P],
                        rhs=xn2T[:, dc, qh * 512:(qh + 1) * 512],
                        start=(dc == 0), stop=(dc == DC - 1))
                nc.scalar.activation(hT[:, m, qh * 512:(qh + 1) * 512], psh,
                                     func=gelu_func, bias=bh_sb[:, m:m + 1],
                                     scale=1.0)

        # load w2 quarters 2-3 into the w1 slots. Sync queue: it is idle from
        # here until the first output store, and at ~280 GB/s the 4MB lands
        # ~14us after FFN1's last read of the w1 quarters frees the slots —
        # just ahead of FFN2 group 0 reaching c=16.
        for g in range(2, 4):
            t = wtile([P, TT, D])
            nc.sync.dma_start(
                t, w2_d[g * 1024:(g + 1) * 1024, :].rearrange(
                    "(o p) f -> p o f", p=P))
            w2_t.append(t)

        xn2Tp.release()

        # ---- phase 6: FFN2 in 4 groups of 2 token tiles (stores overlap) ----
        for g in range(4):
            accs = [sc_tile() for _ in range(4)]   # 2 t-tiles x 2 f-halves
            for c in range(MT):
                for tloc in range(2):
                    ti = g * 2 + tloc
                    for fh in range(2):
                        nc.tensor.matmul(
                            accs[tloc * 2 + fh],
                            lhsT=hT[:, c, ti * P:(ti + 1) * P],
                            rhs=w2_t[c // 8][:, c % 8, fh * 512:(fh + 1) * 512],
                            start=(c == 0), stop=(c == MT - 1))
            for tloc in range(2):
                ti = g * 2 + tloc
                o_t = outp.tile([P, D], f32, tag="o", name=uniq("o"))
                nc.vector.memset(o_t[0:1, 0:4], 0.0)
                for fh in range(2):
                    nc.vector.tensor_add(o_t[:, fh * 512:(fh + 1) * 512],
                                         accs[tloc * 2 + fh],
                                         b2_b[:, fh * 512:(fh + 1) * 512])
                nc.sync.dma_start(out_d[ti * P:(ti + 1) * P, :], o_t)

        w2ep.release()
        hTp.release()
        outp.release()
        wslot.release()
        psum.release()
        stats.release()
        const.release()

    nc.finalize()   # bacc legalization: wait splitting, table/library loads
    return nc


def prep_inputs(inputs):
    """Host-side weight folding + bf16 casts. Returns (shared_map, per_core_x)."""
    f = lambda k: np.asarray(inputs[k], dtype=np.float32)
    x = f("x")
    g1, b1 = f("ln1_g"), f("ln1_b")
    w_qkv, w_out, b_out = f("w_qkv"), f("w_out"), f("b_out")
    g2, b2l = f("ln2_g"), f("ln2_b")
    w1, bias1, w2, bias2 = f("w1"), f("b1"), f("w2"), f("b2")

    scale = DH ** -0.5
    wqkv_g = g1[:, None] * w_qkv
    bias_qkv = b1 @ w_qkv
    bf = ml_dtypes.bfloat16
    shared = {
        "wq": np.ascontiguousarray(wqkv_g[:, :D] * scale).astype(bf),
        "wk": np.ascontiguousarray(wqkv_g[:, D:2 * D]).astype(bf),
        "wv": np.ascontiguousarray(wqkv_g[:, 2 * D:]).astype(bf),
        "wo": w_out.astype(bf),
        "w1": (g2[:, None] * w1).astype(bf),
        "w2": w2.astype(bf),
        "bq": np.ascontiguousarray(bias_qkv[:D] * scale),
        "bk": np.ascontiguousarray(bias_qkv[D:2 * D]),
        "bv": np.ascontiguousarray(bias_qkv[2 * D:]),
        "bo": b_out.copy(),
        "bh": b2l @ w1 + bias1,
        "b2": bias2.copy(),
    }
    xs = [np.ascontiguousarray(x[i]).astype(bf) for i in range(B)]
    return shared, xs


_CACHED_NC = None


def _get_nc():
    global _CACHED_NC
    if _CACHED_NC is None:
        _CACHED_NC = build_bass()
    return _CACHED_NC


def run(inputs, trace=False):
    from concourse.bass_utils import run_bass_kernel_spmd
    nc = _get_nc()
    shared, xs = prep_inputs(inputs)
    in_maps = [{**shared, "x": xs[i]} for i in range(NCORES)]
    res = run_bass_kernel_spmd(nc, in_maps, list(range(NCORES)), trace=trace)
    out = np.stack([np.asarray(res.results[i]["out"]) for i in range(NCORES)], 0)
    return out.astype(np.float32), res


def kernel(**inputs):
    out, _ = run(inputs)
    return out


# revision 11
# speedup vs baseline: 1.5255x; 1.5255x over previous
"""Trainium2 Bass kernel for a dense transformer block (LN -> QKV -> attention ->
out-proj -> LN -> FFN with exact GELU, no residuals).

Sharding: pure data parallelism — batch 8 across 8 NeuronCores, one batch element
per core. Each core runs the full block on its [1024, 1024] token slab.

On-chip dataflow (per core):
  - LN1 in token-major fp32 (bn_stats); normalized output cast to bf16 and
    transposed to feature-major xnT [D, tokens] on the PE (bf16 transpose-mode).
  - All GEMMs in bf16 with fp32 PSUM accumulation. LayerNorm gains fold into the
    weights on the host; LN biases fold into per-feature GEMM biases. The
    1/sqrt(dh) attention scale folds into the Q projection.
  - Q/K produced feature-major per head pair (streamed), V token-major with a
    ones-column per head so attention@V also yields the softmax denominator.
  - Scores are computed k-major (scoresT) so the exp output feeds attention@V
    directly with no transpose; softmax skips max-subtraction (|scores| < ~3).

Scheduling (the perf-critical part — the PE must never idle >3.4us or the HAM
clock gate re-throttles it to 1.2 GHz):
  - DMA priority order at start: x (8 per-tile chunks, so LN1 starts ~1us in and
    the PE transposes keep HAM warm from the start) -> wv -> wq -> wk -> wo.
    All sync-queue DMAs share one HW queue in issue order, ~280 GB/s aggregate.
  - Attention is a single software-pipelined stream per head-pair hp of 32
    "slots" (one score MM each). Interleaved into the slots: the av MMs (lag 4
    behind their exp), the QK chain MMs for hp+1, the denominator finalize for
    the chains that completed (copy den row -> bf16 ones-matmul partition
    broadcast -> DVE reciprocal -> aoT multiply). The PE queue never waits on
    the DVE chain because every consumer sits >=2 slots after its producer.
  - PSUM tags: sc 2x[128,512] (scores/transposes/psb ring), qc 1x[128,1024]
    (QK chains, WAW-rotated), us 4x[128,512] (av accumulators, FFN2
    accumulators) = exactly 8 banks.
  - FFN2 runs in 4 groups of 2 token tiles so output stores overlap compute and
    only ~2us of final DMA is exposed.
"""

import numpy as np
import ml_dtypes
from collections import deque

B, N, D = 8, 1024, 1024
H, DH = 16, 64
MLP = 4096
EPS = 1e-5
P = 128
NCORES = 8
TT = N // P    # 8 token tiles
DC = D // P    # 8 d-chunks
MT = MLP // P  # 32 mlp tiles


def build_bass(gelu_mode="gelu"):
    import concourse.bass as bass
    import concourse.mybir as mybir
    import concourse.tile as tile
    from concourse import bacc
    from concourse.masks import make_identity

    f32 = mybir.dt.float32
    bf16 = mybir.dt.bfloat16
    AF = mybir.ActivationFunctionType
    OP = mybir.AluOpType

    nc = bacc.Bacc()

    x_d = nc.declare_dram_parameter("x", [N, D], bf16, isOutput=False)
    wq_d = nc.declare_dram_parameter("wq", [D, D], bf16, isOutput=False)
    wk_d = nc.declare_dram_parameter("wk", [D, D], bf16, isOutput=False)
    wv_d = nc.declare_dram_parameter("wv", [D, D], bf16, isOutput=False)
    wo_d = nc.declare_dram_parameter("wo", [D, D], bf16, isOutput=False)
    w1_d = nc.declare_dram_parameter("w1", [D, MLP], bf16, isOutput=False)
    w2_d = nc.declare_dram_parameter("w2", [MLP, D], bf16, isOutput=False)
    bq_d = nc.declare_dram_parameter("bq", [D], f32, isOutput=False)
    bk_d = nc.declare_dram_parameter("bk", [D], f32, isOutput=False)
    bv_d = nc.declare_dram_parameter("bv", [D], f32, isOutput=False)
    bo_d = nc.declare_dram_parameter("bo", [D], f32, isOutput=False)
    bh_d = nc.declare_dram_parameter("bh", [MLP], f32, isOutput=False)
    b2_d = nc.declare_dram_parameter("b2", [D], f32, isOutput=False)
    out_d = nc.declare_dram_parameter("out", [N, D], f32, isOutput=True)

    gelu_func = AF.Gelu if gelu_mode == "gelu" else AF.Identity

    with tile.TileContext(nc) as tc:
        # ---- permanent pools (left stack bottom) ----
        const = tc.alloc_tile_pool(name="const", bufs=1)
        stats = tc.alloc_tile_pool(name="stats", bufs=4)
        psum = tc.alloc_tile_pool(name="psum", bufs=2, space="PSUM")
        wslot = tc.alloc_tile_pool(name="wslot", bufs=4)   # 4 x 16KB weight slots
        outp = tc.alloc_tile_pool(name="outp", bufs=2)

        counter = [0]

        def uniq(prefix):
            counter[0] += 1
            return f"{prefix}{counter[0]}"

        def sc_tile(shape=None, dtype=None):
            return psum.tile(shape or [P, 512], dtype or f32, tag="sc", bufs=4,
                             name=uniq("sc"))

        def us_tile():
            return psum.tile([P, 512], f32, tag="us", bufs=2, name=uniq("us"))

        def qc_tile():
            return psum.tile([P, 1024], f32, tag="qc", bufs=1, name=uniq("qc"))

        def wtile(shape):
            return wslot.tile(shape, bf16, tag="w", name=uniq("w"))

        eps_t = const.tile([P, 1], f32, tag="eps")
        nc.vector.memset(eps_t, EPS)
        bq_sb = const.tile([P, DC], f32, tag="bq")
        nc.sync.dma_start(bq_sb, bq_d[:].rearrange("(o p) -> p o", p=P))
        bk_sb = const.tile([P, DC], f32, tag="bk")
        nc.sync.dma_start(bk_sb, bk_d[:].rearrange("(o p) -> p o", p=P))
        bh_sb = const.tile([P, MT], f32, tag="bh")
        nc.sync.dma_start(bh_sb, bh_d[:].rearrange("(o p) -> p o", p=P))
        ident = const.tile([P, P], bf16, tag="ident")
        make_identity(nc, ident)
        ones64b = const.tile([1, 64], bf16, tag="ones64b")
        nc.vector.memset(ones64b, 1.0)

        # ---- phase 1 pools ----
        lnxn = tc.alloc_tile_pool(name="lnxn", bufs=2)     # dies after LN2
        aop = tc.alloc_tile_pool(name="ao", bufs=1)
        uscp = tc.alloc_tile_pool(name="usc", bufs=4)
        denbp = tc.alloc_tile_pool(name="denb", bufs=2)
        x2p = tc.alloc_tile_pool(name="x2", bufs=2)
        xnTp = tc.alloc_tile_pool(name="xnT", bufs=1)
        vap = tc.alloc_tile_pool(name="vaug", bufs=1)
        qkp = tc.alloc_tile_pool(name="qk", bufs=4)
        expp = tc.alloc_tile_pool(name="expp", bufs=8)
        lnx = tc.alloc_tile_pool(name="lnx", bufs=1, side="right")

        # x first among the big DMAs (one chunk per token tile so LN1 can
        # start ~1us in), then weights in order of first use.
        xfull = lnx.tile([P, TT, D], bf16, tag="x")
        for ti in range(TT):
            nc.sync.dma_start(xfull[:, ti, :], x_d[ti * P:(ti + 1) * P, :])
        wv_sb = wtile([P, DC, D])
        nc.sync.dma_start(wv_sb, wv_d[:, :].rearrange("(o p) f -> p o f", p=P))
        wq_sb = wtile([P, DC, D])
        nc.sync.dma_start(wq_sb, wq_d[:, :].rearrange("(o p) f -> p o f", p=P))
        wk_sb = wtile([P, DC, D])
        nc.sync.dma_start(wk_sb, wk_d[:, :].rearrange("(o p) f -> p o f", p=P))
        wo_sb = wtile([P, DC, D])
        nc.sync.dma_start(wo_sb, wo_d[:, :].rearrange("(o p) f -> p o f", p=P))

        # broadcast bias tiles on the gpsimd software queue (parallel path)
        bv_b = const.tile([P, D], bf16, tag="bvb")
        nc.gpsimd.dma_start(bv_b, bv_d[None, :].to_broadcast([P, D]))
        bo_b = const.tile([P, D], bf16, tag="bob")
        nc.gpsimd.dma_start(bo_b, bo_d[None, :].to_broadcast([P, D]))
        b2_b = const.tile([P, D], bf16, tag="b2b")
        nc.gpsimd.dma_start(b2_b, b2_d[None, :].to_broadcast([P, D]))

        def layer_norm_tile(x_t, xn_t):
            """token-major [128, D] -> normalized bf16 (no gain/bias)."""
            nc.vector.memset(xn_t[0:1, 0:4], 0.0)   # claim slot: absorb WAR deps
            st = stats.tile([P, 2, 6], f32, tag="st", name=uniq("st"))
            xr = x_t.rearrange("p (s d) -> p s d", s=2)
            nc.vector.bn_stats(st[:, 0], xr[:, 0])
            nc.vector.bn_stats(st[:, 1], xr[:, 1])
            mv = stats.tile([P, 2], f32, tag="mv", name=uniq("mv"))
            nc.vector.bn_aggr(mv, st)
            rstd = stats.tile([P, 1], f32, tag="rstd", name=uniq("rstd"))
            nc.scalar.activation(rstd, mv[:, 1:2], func=AF.Sqrt, bias=eps_t,
                                 scale=1.0)
            nc.vector.reciprocal(rstd, rstd)
            nc.vector.tensor_scalar(xn_t, x_t, scalar1=mv[:, 0:1], scalar2=rstd,
                                    op0=OP.subtract, op1=OP.mult)

        def pe_transpose_to(dst_of_dj, src_t):
            """[128 tok, D] bf16 -> feature-major dst[:, dj, tok-slice]."""
            for dj in range(DC):
                pst = sc_tile([P, P], bf16)
                nc.tensor.transpose(pst, src_t[:, dj * P:(dj + 1) * P], ident)
                nc.scalar.activation(dst_of_dj(dj), pst, func=AF.Copy)

        # ---- phase 1: LN1 + transpose to feature-major; V fused one behind ----
        xnT = xnTp.tile([P, DC, N], bf16, tag="xnT")
        v_aug = vap.tile([P, TT, H, DH + 1], bf16, tag="vaug")
        nc.vector.memset(v_aug[:, :, :, DH:DH + 1], 1.0)

        def emit_v(ti):
            for fh in range(2):
                psv = us_tile()
                for dc in range(DC):
                    nc.tensor.matmul(psv, lhsT=xnT[:, dc, ti * P:(ti + 1) * P],
                                     rhs=wv_sb[:, dc, fh * 512:(fh + 1) * 512],
                                     start=(dc == 0), stop=(dc == DC - 1))
                nc.vector.tensor_tensor(
                    v_aug[:, ti, fh * 8:(fh + 1) * 8, 0:DH],
                    psv[:].rearrange("p (h d) -> p h d", d=DH),
                    bv_b[:, fh * 512:(fh + 1) * 512].rearrange(
                        "p (h d) -> p h d", d=DH),
                    OP.add)

        for ti in range(TT):
            xn_t = lnxn.tile([P, D], bf16, tag="xn", name=uniq("xn"))
            layer_norm_tile(xfull[:, ti, :], xn_t)
            pe_transpose_to(
                lambda dj, ti=ti: xnT[:, dj, ti * P:(ti + 1) * P], xn_t)
            if ti > 0:
                emit_v(ti - 1)
        emit_v(TT - 1)
        lnx.release()

        # ---- phase 2: attention, software-pipelined slot stream ----
        # Per head-pair hp: 32 score slots (one [128,512] score MM + exp each).
        # The av matmuls trail their exp by LAG slots; the qh0 and qh1 av
        # chains time-share 2 PSUM accumulator banks (qh0 avs end at slot 21,
        # qh1 avs start at slot 22) with a DVE drain to SBUF between tenants.
        # The denominator finalize (bf16 ones-matmul partition broadcast ->
        # reciprocal -> aoT multiply) runs off the SBUF copies, so the PE
        # never waits on it. QK chains for hp+1 are emitted as one dense
        # 6.8us burst at slot 16 — that burst also re-warms the HAM clock
        # gate every iteration if anything throttled the PE.
        aoT = aop.tile([P, DC, N], bf16, tag="aoT")

        qt_of = {}   # hp -> (qt, kt) SBUF tiles
        us_of = {}   # (hp, hh) -> psum accumulator (qh0 then qh1)
        usc_of = {}  # (hp, hh, qh) -> SBUF bf16 copy [DH+1, 512]
        qc_of = {}   # hp -> qc psum tile for the QK chains

        def alloc_qk(hp):
            qt = qkp.tile([P, N], bf16, tag="qT", name=uniq("qT"))
            kt = qkp.tile([P, N], bf16, tag="kT", name=uniq("kT"))
            nc.vector.memset(qt[0:1, 0:4], 0.0)
            nc.vector.memset(kt[0:1, 0:4], 0.0)
            qt_of[hp] = (qt, kt)
            qc_of[hp] = qc_tile()

        # chain c: 0=q/qh0, 1=k/qh0, 2=q/qh1, 3=k/qh1. qc[:, 0:512] holds q
        # chains, qc[:, 512:1024] k chains (WAW rotated after the DVE add).
        def emit_qk_block(hp):
            qt, kt = qt_of[hp]
            qc = qc_of[hp]
            for c in range(4):
                w_sb = wq_sb if c % 2 == 0 else wk_sb
                qh = c // 2
                half = qc[:, (c % 2) * 512:(c % 2) * 512 + 512]
                for j in range(DC):
                    nc.tensor.matmul(half,
                                     lhsT=w_sb[:, j, hp * P:(hp + 1) * P],
                                     rhs=xnT[:, j, qh * 512:(qh + 1) * 512],
                                     start=(j == 0), stop=(j == DC - 1))
                dst = qt if c % 2 == 0 else kt
                bias = bq_sb if c % 2 == 0 else bk_sb
                nc.vector.tensor_scalar_add(dst[:, qh * 512:(qh + 1) * 512],
                                            half, bias[:, hp:hp + 1])

        pend = deque()   # (hp, hh, qh, mc, ext)
        LAG = 6

        def flush_av():
            hp_, hh, qh, mc, ext = pend.popleft()
            nc.tensor.matmul(us_of[(hp_, hh)][0:DH + 1, :],
                             lhsT=v_aug[:, mc, 2 * hp_ + hh, :],
                             rhs=ext,
                             start=(mc == 0), stop=(mc == TT - 1))

        def drain_us(hp, qh):
            for hh in range(2):
                u = uscp.tile([DH, 512], bf16, tag="usc", name=uniq("usc"))
                nc.vector.tensor_copy(u, us_of[(hp, hh)][0:DH, :])
                dn = uscp.tile([1, 512], bf16, tag="den", name=uniq("den"))
                nc.vector.tensor_copy(dn, us_of[(hp, hh)][DH:DH + 1, :])
                usc_of[(hp, hh, qh)] = (u, dn)

        def fin(hp, qh):
            for hh in range(2):
                u, dn = usc_of.pop((hp, hh, qh))
                psb = sc_tile()
                nc.tensor.matmul(psb[0:64, :], lhsT=ones64b[:],
                                 rhs=dn, start=True, stop=True)
                dB = denbp.tile([64, 512], f32, tag="dB", name=uniq("dB"))
                # softmax denominators are sums of exps in ~[10, 2e3] — safe
                # for the approx op; ~18 correct bits vs the bf16 math around
                nc.vector.reciprocal_approx_fast(dB, psb[0:64, :])
                nc.vector.tensor_mul(
                    aoT[64 * hh:64 * hh + 64, hp, qh * 512:(qh + 1) * 512],
                    u, dB)

        # pre-loop: QK chains for hp=0 run right after the V matmuls
        alloc_qk(0)
        emit_qk_block(0)

        score_order = [(qh, mc, hh) for qh in range(2) for mc in range(TT)
                       for hh in range(2)]

        for hp in range(DC):
            us_of[(hp, 0)] = us_tile()
            us_of[(hp, 1)] = us_tile()
            if hp < DC - 1:
                alloc_qk(hp + 1)
            qt, kt = qt_of[hp]
            for s, (qh, mc, hh) in enumerate(score_order):
                r0 = 64 * hh
                sp = sc_tile()
                nc.tensor.matmul(sp,
                                 lhsT=kt[r0:r0 + 64, mc * P:(mc + 1) * P],
                                 rhs=qt[r0:r0 + 64, qh * 512:(qh + 1) * 512],
                                 start=True, stop=True)
                ext = expp.tile([P, 512], bf16, tag="expT", name=uniq("expT"))
                nc.scalar.activation(ext, sp, func=AF.Exp)
                pend.append((hp, hh, qh, mc, ext))
                if s == 6 and hp > 0:
                    drain_us(hp - 1, 1)
                if s == 8 and hp > 0:
                    fin(hp - 1, 1)
                if s == 16 and hp < DC - 1:
                    emit_qk_block(hp + 1)
                if s == 22:
                    drain_us(hp, 0)
                if s == 24:
                    fin(hp, 0)
                while len(pend) > LAG:
                    flush_av()
        while pend:
            flush_av()
        drain_us(DC - 1, 1)
        fin(DC - 1, 1)

        expp.release()
        qkp.release()
        vap.release()
        xnTp.release()

        # q/k/v slots free -> start w1 loads
        w1_t = []
        for g in range(3):   # quarters 0-2 reuse the q/k/v slots right away
            t = wtile([P, 2, MLP])
            nc.sync.dma_start(
                t, w1_d[g * 256:(g + 1) * 256, :].rearrange(
                    "(o p) f -> p o f", p=P))
            w1_t.append(t)

        # ---- phase 4: out-projection + LN2 + transpose ----
        hTp = tc.alloc_tile_pool(name="hT", bufs=1, side="right")
        xn2Tp = tc.alloc_tile_pool(name="xn2T", bufs=1, side="right")
        hT = hTp.tile([P, MT, N], bf16, tag="hT")
        xn2T = xn2Tp.tile([P, DC, N], bf16, tag="xn2T")

        pend_tr = [None]
        for ti in range(TT):
            x2_t = x2p.tile([P, D], bf16, tag="x2", name=uniq("x2"))
            nc.vector.memset(x2_t[0:1, 0:4], 0.0)
            for fh in range(2):
                pso = sc_tile()
                for dc in range(DC):
                    nc.tensor.matmul(pso, lhsT=aoT[:, dc, ti * P:(ti + 1) * P],
                                     rhs=wo_sb[:, dc, fh * 512:(fh + 1) * 512],
                                     start=(dc == 0), stop=(dc == DC - 1))
                nc.vector.tensor_add(x2_t[:, fh * 512:(fh + 1) * 512], pso,
                                     bo_b[:, fh * 512:(fh + 1) * 512])
            # defer this tile's transposes behind the next tile's out-proj so
            # the in-order PE queue never waits on the DVE LayerNorm chain
            if pend_tr[0] is not None:
                pend_tr[0]()
            xn2_t = lnxn.tile([P, D], bf16, tag="xn", name=uniq("xn"))
            layer_norm_tile(x2_t, xn2_t)
            pend_tr[0] = (lambda ti=ti, xt=xn2_t: pe_transpose_to(
                lambda dj: xn2T[:, dj, ti * P:(ti + 1) * P], xt))
        pend_tr[0]()

        # wout done -> load the last w1 quarter into its slot
        w1d = wtile([P, 2, MLP])
        nc.sync.dma_start(
            w1d, w1_d[768:1024, :].rearrange("(o p) f -> p o f", p=P))
        w1_t.append(w1d)

        x2p.release()
        denbp.release()
        uscp.release()
        aop.release()
        lnxn.release()
        w2ep = tc.alloc_tile_pool(name="w2e", bufs=2)
        w2_t = []
        for g in range(2):
            t = w2ep.tile([P, TT, D], bf16, tag="w2e", name=uniq("w2e"))
            nc.sync.dma_start(
                t, w2_d[g * 1024:(g + 1) * 1024, :].rearrange(
                    "(o p) f -> p o f", p=P))
            w2_t.append(t)

        # ---- phase 5: FFN1 (feature-major h, fused bias+gelu) ----
        for m in range(MT):
            for qh in range(2):
                psh = sc_tile()
                for dc in range(DC):
                    nc.tensor.matmul(
                        psh,
                        lhsT=w1_t[dc // 2][:, dc % 2, m * P:(m + 1) * P],
                        rhs=xn2T[:, dc, qh * 512:(qh + 1) * 512],
                        start=(dc == 0), stop=(dc == DC - 1))
                nc.scalar.activation(hT[:, m, qh * 512:(qh + 1) * 512], psh,
                                     func=gelu_func, bias=bh_sb[:, m:m + 1],
                                     scale=1.0)

        # load w2 quarters 2-3 into the w1 slots. Sync queue: it is idle from
        # here until the first output store, and at ~280 GB/s the 4MB lands
        # ~14us after FFN1's last read of the w1 quarters frees the slots —
        # just ahead of FFN2 group 0 reaching c=16.
        for g in range(2, 4):
            t = wtile([P, TT, D])
            nc.sync.dma_start(
                t, w2_d[g * 1024:(g + 1) * 1024, :].rearrange(
                    "(o p) f -> p o f", p=P))
            w2_t.append(t)

        xn2Tp.release()

        # ---- phase 6: FFN2 in 4 groups of 2 token tiles (stores overlap) ----
        for g in range(4):
            accs = [sc_tile() for _ in range(4)]   # 2 t-tiles x 2 f-halves
            for c in range(MT):
                for tloc in range(2):
                    ti = g * 2 + tloc
                    for fh in range(2):
                        nc.tensor.matmul(
                            accs[tloc * 2 + fh],
                            lhsT=hT[:, c, ti * P:(ti + 1) * P],
                            rhs=w2_t[c // 8][:, c % 8, fh * 512:(fh + 1) * 512],
                            start=(c == 0), stop=(c == MT - 1))
            for tloc in range(2):
                ti = g * 2 + tloc
                o_t = outp.tile([P, D], f32, tag="o", name=uniq("o"))
                nc.vector.memset(o_t[0:1, 0:4], 0.0)
                for fh in range(2):
                    nc.vector.tensor_add(o_t[:, fh * 512:(fh + 1) * 512],
                                         accs[tloc * 2 + fh],
                                         b2_b[:, fh * 512:(fh + 1) * 512])
                nc.sync.dma_start(out_d[ti * P:(ti + 1) * P, :], o_t)

        w2ep.release()
        hTp.release()
        outp.release()
        wslot.release()
        psum.release()
        stats.release()
        const.release()

    nc.finalize()   # bacc legalization: wait splitting, table/library loads
    return nc


def prep_inputs(inputs):
    """Host-side weight folding + bf16 casts. Returns (shared_map, per_core_x)."""
    f = lambda k: np.asarray(inputs[k], dtype=np.float32)
    x = f("x")
    g1, b1 = f("ln1_g"), f("ln1_b")
    w_qkv, w_out, b_out = f("w_qkv"), f("w_out"), f("b_out")
    g2, b2l = f("ln2_g"), f("ln2_b")
    w1, bias1, w2, bias2 = f("w1"), f("b1"), f("w2"), f("b2")

    scale = DH ** -0.5
    wqkv_g = g1[:, None] * w_qkv
    bias_qkv = b1 @ w_qkv
    bf = ml_dtypes.bfloat16
    shared = {
        "wq": np.ascontiguousarray(wqkv_g[:, :D] * scale).astype(bf),
        "wk": np.ascontiguousarray(wqkv_g[:, D:2 * D]).astype(bf),
        "wv": np.ascontiguousarray(wqkv_g[:, 2 * D:]).astype(bf),
        "wo": w_out.astype(bf),
        "w1": (g2[:, None] * w1).astype(bf),
        "w2": w2.astype(bf),
        "bq": np.ascontiguousarray(bias_qkv[:D] * scale),
        "bk": np.ascontiguousarray(bias_qkv[D:2 * D]),
        "bv": np.ascontiguousarray(bias_qkv[2 * D:]),
        "bo": b_out.copy(),
        "bh": b2l @ w1 + bias1,
        "b2": bias2.copy(),
    }
    xs = [np.ascontiguousarray(x[i]).astype(bf) for i in range(B)]
    return shared, xs


_CACHED_NC = None


def _get_nc():
    global _CACHED_NC
    if _CACHED_NC is None:
        _CACHED_NC = build_bass()
    return _CACHED_NC


def run(inputs, trace=False):
    from concourse.bass_utils import run_bass_kernel_spmd
    nc = _get_nc()
    shared, xs = prep_inputs(inputs)
    in_maps = [{**shared, "x": xs[i]} for i in range(NCORES)]
    res = run_bass_kernel_spmd(nc, in_maps, list(range(NCORES)), trace=trace)
    out = np.stack([np.asarray(res.results[i]["out"]) for i in range(NCORES)], 0)
    return out.astype(np.float32), res


def kernel(**inputs):
    out, _ = run(inputs)
    return out
